# revision 11
# baseline (speedup 1.0000x reference)
"""MinGRU Trainium2 kernel.

Reference computation (per batch element b, sequence length T, hidden H):
    k  = x @ W_z + b_z                       # [T, H]
    th = x @ W_h + b_h                       # [T, H]
    z  = sigmoid(k);  a = sigmoid(-k) = 1 - z
    g  = where(th >= 0, th + 0.5, sigmoid(th)) == max(th + 0.5, sigmoid(th))
    h[t] = a[t] * h[t-1] + z[t] * g[t]       # linear scan along T
Output h  # [B, T, H]

Strategy: data-parallel over batch (B=8 -> 8 NeuronCores). Host transposes
x[b] to [D, T] (bf16) so both matmuls produce [H, T] tiles directly
(contraction dim D on partitions; W is already the lhsT layout [D, H],
bf16 -> automatic Fast Weight Load). Engine split of the elementwise tail
(per [128, 1024] chunk):
    Act:    sg = sigmoid(th), z = sigmoid(k), a = sigmoid(-k)   (3 ops)
    DVE:    u  = max(th + 0.5, sg)  [scalar_tensor_tensor, reads PSUM]
            h  = scan(a, b): state = a*state + b   [TENSOR_TENSOR_SCAN]
    GpSimd: b  = z * u              [tensor_tensor mult; Pool has no
                                     STT/scan/max support, mult only]
Scan emission is software-pipelined one chunk behind so the DVE FIFO never
stalls on the GpSimd mult. Output h is written bf16 and up-converted /
transposed on host (rel tolerance 2e-2; bf16 path measures ~4e-3).
"""

import numpy as np

B, T, D, H = 8, 4096, 512, 512
N_CORES = 8
MMN = 512                 # matmul free dim (PSUM bank limit for fp32)
TCH = 1024                # PSUM / elementwise / scan chunk along T
NT = T // TCH             # 4
NM = H // 128             # 4 partition tiles of H
NK = D // 128             # 4 contraction tiles

_cache = {}


def _build():
    import concourse.tile as tile
    from concourse import bacc, mybir

    f32 = mybir.dt.float32
    bf16 = mybir.dt.bfloat16
    AF = mybir.ActivationFunctionType
    ALU = mybir.AluOpType

    nc = bacc.Bacc("TRN2", target_bir_lowering=False, debug=False,
                   num_devices=N_CORES)

    xt_d = nc.dram_tensor("xt", [D, T], bf16, kind="ExternalInput").ap()
    wz_d = nc.dram_tensor("wz", [D, H], bf16, kind="ExternalInput").ap()
    wh_d = nc.dram_tensor("wh", [D, H], bf16, kind="ExternalInput").ap()
    bias_d = nc.dram_tensor("bias", [128, 4 * NM], f32,
                            kind="ExternalInput").ap()
    ht_d = nc.dram_tensor("ht", [H, T], bf16, kind="ExternalOutput").ap()

    with tile.TileContext(nc) as tc:
        with (
            tc.tile_pool(name="const", bufs=1) as const,
            tc.tile_pool(name="chunks", bufs=4) as chunks,
            tc.tile_pool(name="psum", bufs=2, space="PSUM") as psum,
        ):
            # wh needed first (psT matmuls + warm-up); split weight loads
            # across the sync and scalar rings so transfers overlap; x on
            # the gpsimd ring.
            bias_s = const.tile([128, 4 * NM], f32, tag="bias")
            nc.sync.dma_start(bias_s[:], bias_d[:])
            # weights as half-tiles so the PE warm-up (and the first psT
            # matmuls) gate only on the first half landing, with the two
            # halves transferring on separate rings concurrently
            wh_r = wh_d.rearrange("(k p) h -> p k h", p=128)
            wh01 = const.tile([128, 2, H], bf16, tag="wh01")
            nc.sync.dma_start(wh01[:], wh_r[:, 0:2])
            wh23 = const.tile([128, 2, H], bf16, tag="wh23")
            nc.scalar.dma_start(wh23[:], wh_r[:, 2:4])
            wz_r = wz_d.rearrange("(k p) h -> p k h", p=128)
            wz01 = const.tile([128, 2, H], bf16, tag="wz01")
            nc.sync.dma_start(wz01[:], wz_r[:, 0:2])
            wz23 = const.tile([128, 2, H], bf16, tag="wz23")
            nc.scalar.dma_start(wz23[:], wz_r[:, 2:4])
            wh_k = (wh01, wh01, wh23, wh23)
            wz_k = (wz01, wz01, wz23, wz23)
            xt_s = const.tile([128, NK, T], bf16, tag="xt")
            xt_r = xt_d.rearrange("(k p) t -> p k t", p=128)
            nc.gpsimd.dma_start(xt_s[:, :, 0:MMN], xt_r[:, :, 0:MMN])
            nc.gpsimd.dma_start(xt_s[:, :, MMN:TCH], xt_r[:, :, MMN:TCH])
            for tc_i in range(1, NT):
                tsl = slice(tc_i * TCH, (tc_i + 1) * TCH)
                nc.gpsimd.dma_start(xt_s[:, :, tsl], xt_r[:, :, tsl])

            # PE warm-up during the x DMA wait: dummy matmuls on wh data
            # (first DMA to land) so the HAM clock gate is at full rate
            # when real work arrives.
            warm = psum.tile([128, TCH], f32, tag="psK")
            for r in range(3):
                nc.tensor.matmul(warm[:, 0:MMN], wh01[:, 0, 0:128],
                                 wh01[:, 0, 0:MMN], start=True, stop=True)

            # Software-pipelined: iteration ci issues matmuls/act/u/b for
            # chunk ci and the scan+output-DMA for chunk ci-2 (lag 2 keeps
            # the strict-FIFO DVE busy with u(i+1) while GpSimd computes
            # b(i) ahead of scan(i)).
            LAG = 2
            pend = []            # [(m, tc_i, a, b), ...] awaiting scan
            h_prev = None        # previous h tile of the current m chain
            for ci in range(NM * NT + LAG):
                # scan(ci-LAG) is emitted BEFORE u(ci) so the strict-FIFO
                # DVE never head-of-line-blocks a ready scan behind a u
                # that still waits on its sg.
                if ci >= LAG:
                    pm, ptc, pa, pbb = pend.pop(0)
                    # h[t] = a[t]*h[t-1] + b[t], chained across chunks of m.
                    # tile_wait_until pins the scan ~2 chunk-cycles behind
                    # its producers in the scheduler's simulation; without
                    # it the scheduler re-packs the DVE queue to lag 1 and
                    # the DVE idles a full gpsimd-mult per chunk.
                    h = chunks.tile([128, TCH], bf16, tag="h", bufs=5)
                    init = 0.0 if ptc == 0 else h_prev[:, TCH - 1:TCH]
                    with tc.tile_wait_until((16.0 + 4.0 * (ci - LAG)) / 1000.0):
                        nc.vector.tensor_tensor_scan(h[:], pa[:], pbb[:], init,
                                                     ALU.mult, ALU.add)
                        nc.sync.dma_start(
                            ht_d[pm * 128:(pm + 1) * 128,
                                 ptc * TCH:(ptc + 1) * TCH], h[:])
                    h_prev = h
                if ci < NM * NT:
                    m, tc_i = divmod(ci, NT)
                    nbz = bias_s[:, 0 * NM + m:0 * NM + m + 1]
                    bh = bias_s[:, 1 * NM + m:1 * NM + m + 1]
                    bh5 = bias_s[:, 2 * NM + m:2 * NM + m + 1]
                    pbz = bias_s[:, 3 * NM + m:3 * NM + m + 1]
                    psT = psum.tile([128, TCH], f32, tag="psT")
                    psK = psum.tile([128, TCH], f32, tag="psK")
                    msl = slice(m * 128, (m + 1) * 128)
                    # all psT groups first so sg (and the u -> b -> scan
                    # chain behind it) starts as early as possible
                    for ps, w_k in ((psT, wh_k), (psK, wz_k)):
                        for sub in range(TCH // MMN):
                            nsl = slice(tc_i * TCH + sub * MMN,
                                        tc_i * TCH + (sub + 1) * MMN)
                            osl = slice(sub * MMN, (sub + 1) * MMN)
                            for k in range(NK):
                                nc.tensor.matmul(ps[:, osl],
                                                 w_k[k][:, k % 2, msl],
                                                 xt_s[:, k, nsl],
                                                 start=(k == 0),
                                                 stop=(k == NK - 1))
                    # sg = sigmoid(th0 + b_h)
                    sg = chunks.tile([128, TCH], bf16, tag="sg", bufs=3)
                    nc.scalar.activation(sg[:], psT[:], AF.Sigmoid,
                                         bias=bh, scale=1.0)
                    # z = sigmoid(k0 + b_z)
                    z = chunks.tile([128, TCH], bf16, tag="z", bufs=3)
                    nc.scalar.activation(z[:], psK[:], AF.Sigmoid,
                                         bias=pbz, scale=1.0)
                    # a = sigmoid(-(k0 + b_z)) = 1 - z
                    a = chunks.tile([128, TCH], bf16, tag="a", bufs=6)
                    nc.scalar.activation(a[:], psK[:], AF.Sigmoid,
                                         bias=nbz, scale=-1.0)
                    # u = max(th0 + (b_h + 0.5), sg)   (= g, the candidate)
                    u = chunks.tile([128, TCH], bf16, tag="u", bufs=3)
                    nc.vector.scalar_tensor_tensor(
                        u[:], psT[:], bh5, sg[:], ALU.add, ALU.max)
                    # b = z * u, on GpSimd (SBUF-only inputs, plain mult)
                    bb = chunks.tile([128, TCH], bf16, tag="bb", bufs=5)
                    beng = nc.vector if ci == NM * NT - 1 else nc.gpsimd
                    beng.tensor_tensor(bb[:], z[:], u[:], ALU.mult)
                    pend.append((m, tc_i, a, bb))

    nc.compile()
    return nc


def kernel(x, W_z, b_z, W_h, b_h):
    import ml_dtypes
    from concourse.bass_utils import run_bass_kernel_spmd

    if "nc" not in _cache:
        _cache["nc"] = _build()
    nc = _cache["nc"]

    bf = ml_dtypes.bfloat16
    x = np.asarray(x, dtype=np.float32)
    W_z = np.ascontiguousarray(np.asarray(W_z, dtype=np.float32).astype(bf))
    W_h = np.ascontiguousarray(np.asarray(W_h, dtype=np.float32).astype(bf))
    b_z = np.asarray(b_z, dtype=np.float32)
    b_h = np.asarray(b_h, dtype=np.float32)

    nbz = (-b_z).reshape(NM, 128).T
    bh = b_h.reshape(NM, 128).T
    bh5 = (b_h + 0.5).reshape(NM, 128).T
    pbz = b_z.reshape(NM, 128).T
    bias = np.ascontiguousarray(
        np.concatenate([nbz, bh, bh5, pbz], axis=1).astype(np.float32))

    in_maps = []
    for b in range(B):
        in_maps.append({
            "xt": np.ascontiguousarray(x[b].T.astype(bf)),
            "wz": W_z,
            "wh": W_h,
            "bias": bias,
        })

    import os
    kwargs = {}
    if os.environ.get("KERNEL_TRACE"):
        kwargs = dict(trace=True, tmpdir=os.environ.get("KERNEL_TMPDIR"))
    try:
        res = run_bass_kernel_spmd(nc, in_maps, core_ids=list(range(N_CORES)),
                                   **kwargs)
    except Exception:
        # transient accelerator errors recover on retry
        res = run_bass_kernel_spmd(nc, in_maps, core_ids=list(range(N_CORES)),
                                   **kwargs)
    _cache["last_results"] = res

    out = np.empty((B, T, H), dtype=np.float32)
    for b in range(B):
        out[b] = np.asarray(res.results[b]["ht"]).astype(np.float32).T
    return out
